# revision 1
# baseline (speedup 1.0000x reference)
"""Trainium2 Bass kernel for Conv2D (1x1) multi-head attention block.

Reference computation (per batch image of [64, 64, 512] = [N=4096, C=512]):
    x  = GroupNorm(inputs, G=32, eps=1e-6) * gamma + beta
    q, k, v = x @ wq + bq, x @ wk + bk, x @ wv + bv      (1x1 convs)
    scores  = (q / sqrt(C)) @ k^T                         [N, N]
    out     = softmax(scores) @ v @ wo + bo + inputs

Sharding: 8 cores = 2 batches x 4 query-quarters. Each core holds the full
image of its batch (needed for GroupNorm stats and full-attention K/V) and
computes the output rows of its query quarter only.  No collectives: the
redundant K/V compute is cheaper than a DRAM-bounce AllGather here.

Key implementation choices:
  - GroupNorm is folded into the projection weights: with per-channel
    a[c] = gamma*rstd, b[c] = beta - mean*gamma*rstd, we have
    K^T = (diag(a) wk)^T x^T + (wk^T b + bk) 1^T, so normalized
    activations are never materialized.  Stats come from ones-matmuls
    (per-channel sum / sum-of-squares) in float32r (TF32-like) during the
    single streaming pass over x.
  - x^T is produced once by PE transposes of 128x128 blocks and kept
    resident in bf16; K^T, Q^T, V and all attention matmuls run in bf16
    (fp32 PSUM accumulation).  bf16 weights get fast-weight-load, which
    roughly halves the per-matmul cost vs 4-byte dtypes.  The residual add
    and all softmax normalization stay fp32, and the attention output is
    only ~4% of the output magnitude, so end-to-end error stays ~2e-4.
  - Attention uses the transposed-scores layout: scores^T[k, q] tiles come
    from matmul(lhsT=K^T tile, rhs=Q^T chunk); exp runs on the scalar
    engine PSUM->SBUF (no max-subtraction: scores are O(1) by construction
    since q is pre-scaled by 1/sqrt(C)); probs^T feeds
    matmul(lhsT=V tile, rhs=probs^T) accumulating attn^T[c, q] in PSUM over
    all 32 key tiles, and a ones-column matmul accumulates the softmax
    denominators.  The output projection consumes the *unnormalized*
    attn^T immediately; 1/rowsum is applied per-partition at the final
    PSUM->SBUF copy, keeping the PE free of the softmax epilogue.
"""

import sys

sys.path.insert(0, "/opt/trn_rl_repo")

from contextlib import ExitStack

import numpy as np

import concourse.bacc as bacc
import concourse.tile as tile
from concourse import mybir
from concourse.bass_utils import run_bass_kernel_spmd

# Problem shape (hardcoded; kernel.py must be self-contained).
B, HH, WW, C = 2, 64, 64, 512
N = HH * WW          # 4096 pixels per batch image
G = 32               # groupnorm groups
GS = C // G          # 16 channels per group
EPS = 1e-6
P = 128              # partitions
CT = C // P          # 4 channel tiles
NT = N // P          # 32 pixel tiles per image
CHUNK = 512          # free-dim chunk for moving operands
NCH = N // CHUNK     # 8 pixel chunks per image
NCORES = 8
QS = N // 4          # 1024 query rows per core
QTILES = QS // P     # 8 query tiles per core
QCH = QS // CHUNK    # 2 query chunks per core
GROUP_COUNT = N * GS  # elements per (batch, group) for the mean/var

F32 = mybir.dt.float32
F32R = mybir.dt.float32r
BF16 = mybir.dt.bfloat16
AF = mybir.ActivationFunctionType

_NC_CACHE = None


def _build():
    nc = bacc.Bacc(None, target_bir_lowering=False, debug=False)

    # x arrives pre-cast to bf16 (host-side layout prep, like the
    # per-core sharding); x^T is built by hardware DMA-transpose reads
    # straight from DRAM, so the PE does no transposes at all.  The fp32
    # copy of the query quarter feeds the residual add.
    x_bf = nc.dram_tensor("x_bf", [N, C], BF16, kind="ExternalInput")
    x_resq_bf = nc.dram_tensor("x_resq_bf", [QS, C], BF16, kind="ExternalInput")
    x_res = nc.dram_tensor("x_res", [QS, C], F32, kind="ExternalInput")
    gamma_d = nc.dram_tensor("gamma", [C], F32, kind="ExternalInput")
    beta_d = nc.dram_tensor("beta", [C], F32, kind="ExternalInput")
    w_d = {}
    b_d = {}
    for nm in ("wq", "wk", "wv", "wo"):
        w_d[nm] = nc.dram_tensor(nm, [C, C], F32, kind="ExternalInput")
    for nm in ("bq", "bk", "bv", "bo"):
        b_d[nm] = nc.dram_tensor(nm, [C], F32, kind="ExternalInput")
    ident_d = nc.dram_tensor("ident", [P, P], F32R, kind="ExternalInput")
    gind_d = nc.dram_tensor("gind", [P, 8], F32, kind="ExternalInput")
    gindt_d = nc.dram_tensor("gindt", [8, P], F32, kind="ExternalInput")
    out_d = nc.dram_tensor("out", [QS, C], F32, kind="ExternalOutput")

    with tile.TileContext(nc) as tc, ExitStack() as top:
        # ---- persistent pools ----
        consts = top.enter_context(tc.tile_pool(name="consts", bufs=1))
        pkt = top.enter_context(tc.tile_pool(name="pkt", bufs=1))
        pqt = top.enter_context(tc.tile_pool(name="pqt", bufs=1))
        pv = top.enter_context(tc.tile_pool(name="pv", bufs=1))
        pxt = top.enter_context(tc.tile_pool(name="pxt", bufs=1))
        pmisc = top.enter_context(tc.tile_pool(name="pmisc", bufs=1))

        ident = consts.tile([P, P], F32R, name="ident")
        nc.sync.dma_start(out=ident, in_=ident_d[:])
        gind = consts.tile([P, 8], F32, name="gind")
        nc.sync.dma_start(out=gind, in_=gind_d[:])
        gindt = consts.tile([8, P], F32, name="gindt")
        nc.sync.dma_start(out=gindt, in_=gindt_d[:])
        ones_f32 = consts.tile([P, 1], F32, name="ones_f32")
        nc.vector.memset(ones_f32, 1.0)
        ones_bf = consts.tile([P, 1], BF16, name="ones_bf")
        nc.scalar.copy(ones_bf, ones_f32)
        one11 = ones_f32[0:1, 0:1]

        gamma4, beta4 = [], []
        for ct in range(CT):
            gt_ = consts.tile([P, 1], F32, name=f"gamma4_{ct}")
            nc.sync.dma_start(out=gt_, in_=gamma_d[ct * P:(ct + 1) * P])
            gamma4.append(gt_)
            bt_ = consts.tile([P, 1], F32, name=f"beta4_{ct}")
            nc.sync.dma_start(out=bt_, in_=beta_d[ct * P:(ct + 1) * P])
            beta4.append(bt_)

        # Resident activations: x^T, K^T, Q^T, V natural -- all bf16
        xt = [pxt.tile([P, N], BF16, name=f"xt{i}", tag=f"xt{i}") for i in range(CT)]
        kt = [pkt.tile([P, N], BF16, name=f"kt{i}", tag=f"kt{i}") for i in range(CT)]
        qt = [pqt.tile([P, QS], BF16, name=f"qt{i}", tag=f"qt{i}") for i in range(CT)]
        vv = [pv.tile([P, C], BF16, name=f"v{i}", tag=f"v{i}") for i in range(NT)]
        # x^T of the query quarter (for Q projection)
        xtq = [pxt.tile([P, QS], BF16, name=f"xtq{i}", tag=f"xtq{i}")
               for i in range(CT)]

        with ExitStack() as dphase:
            psp = dphase.enter_context(tc.tile_pool(name="psp", bufs=3, space="PSUM"))

            # per-channel bn_stats accumulators, one [P, NCH, 6] per ct
            bnst = [pmisc.tile([P, NCH, 6], F32, name=f"bnst{i}")
                    for i in range(CT)]

            # Warm-keeper: idle-PE filler matmuls so the HAM clock gate
            # stays at full rate while DMA/DVE do the x^T build.
            warm32 = pmisc.tile([P, CHUNK], F32, name="warm32")
            nc.vector.memset(warm32, 1.0)
            warm_src = pmisc.tile([P, CHUNK], F32R, name="warm_src")
            nc.scalar.copy(warm_src, warm32)

            def keep_warm(n):
                for _ in range(n):
                    wps = psp.tile([P, CHUNK], F32, name="wps", tag="kps")
                    nc.tensor.matmul(wps, lhsT=ident, rhs=warm_src,
                                     start=True, stop=True)

            # ==== Phase A: build x^T via hardware DMA-transpose ====
            for ch in range(NCH):
                for ct in range(CT):
                    nc.sync.dma_start_transpose(
                        xt[ct][:, ch * CHUNK:(ch + 1) * CHUNK],
                        x_bf[ch * CHUNK:(ch + 1) * CHUNK, ct * P:(ct + 1) * P])
                    nc.vector.bn_stats(
                        out=bnst[ct][:, ch, :],
                        in_=xt[ct][:, ch * CHUNK:(ch + 1) * CHUNK])
                keep_warm(3)
            for ch in range(QCH):
                for ct in range(CT):
                    nc.sync.dma_start_transpose(
                        xtq[ct][:, ch * CHUNK:(ch + 1) * CHUNK],
                        x_resq_bf[ch * CHUNK:(ch + 1) * CHUNK,
                                  ct * P:(ct + 1) * P])
                keep_warm(3)

            # ==== Phase B: group stats -> per-channel a, b (partition-major)
            a4, aq4, b4 = [], [], []
            with tc.tile_pool(name="psb", bufs=1, space="PSUM") as psb, \
                 tc.tile_pool(name="pb", bufs=2) as pb:
                for ct in range(CT):
                    mv = pb.tile([P, 2], F32, name="mv", tag="mv")
                    nc.vector.bn_aggr(out=mv, in_=bnst[ct])
                    # per-channel (mean, E[x^2])
                    me2 = pb.tile([P, 2], F32, name="me2", tag="me2")
                    nc.vector.tensor_copy(me2[:, 0:1], mv[:, 0:1])
                    nc.vector.tensor_mul(me2[:, 1:2], mv[:, 0:1], mv[:, 0:1])
                    nc.vector.tensor_add(me2[:, 1:2], me2[:, 1:2], mv[:, 1:2])
                    keep_warm(6)
                    grp_ps = psb.tile([8, 2], F32, name="grp_ps", tag="grp_ps")
                    nc.tensor.matmul(grp_ps, lhsT=gind, rhs=me2,
                                     start=True, stop=True)
                    grp = pb.tile([8, 2], F32, name="grp", tag="grp")
                    nc.vector.tensor_scalar_mul(grp, grp_ps, 1.0 / GS)
                    var = pb.tile([8, 1], F32, name="var", tag="var")
                    nc.vector.tensor_mul(var, grp[:, 0:1], grp[:, 0:1])
                    nc.vector.tensor_sub(var, grp[:, 1:2], var)
                    nc.vector.tensor_scalar_add(var, var, EPS)
                    rstd = pb.tile([8, 1], F32, name="rstd", tag="rstd")
                    nc.vector.reciprocal(rstd, var)
                    nc.scalar.sqrt(rstd, rstd)
                    mr = pb.tile([8, 2], F32, name="mr", tag="mr")
                    nc.vector.tensor_copy(mr[:, 0:1], grp[:, 0:1])
                    nc.vector.tensor_copy(mr[:, 1:2], rstd)
                    mch_ps = psb.tile([P, 2], F32, name="mch_ps", tag="mch_ps")
                    nc.tensor.matmul(mch_ps, lhsT=gindt, rhs=mr,
                                     start=True, stop=True)
                    keep_warm(6)
                    mch = pb.tile([P, 2], F32, name="mch", tag="mch")
                    nc.vector.tensor_copy(mch, mch_ps)
                    a_t = pmisc.tile([P, 1], F32, name=f"a4_{ct}")
                    nc.vector.tensor_mul(a_t, gamma4[ct], mch[:, 1:2])
                    a4.append(a_t)
                    aq_t = pmisc.tile([P, 1], F32, name=f"aq4_{ct}")
                    nc.vector.tensor_scalar_mul(aq_t, a_t, 1.0 / float(np.sqrt(C)))
                    aq4.append(aq_t)
                    b_t = pmisc.tile([P, 1], F32, name=f"b4_{ct}")
                    nc.vector.tensor_mul(b_t, mch[:, 0:1], a_t)
                    nc.vector.tensor_sub(b_t, beta4[ct], b_t)
                    b4.append(b_t)

            # ==== Phase C: fold weights (bf16) + biases ====
            def fold_weight(nm, scales, qscale, pool, pspool, wpool):
                wf, raws = [], []
                for ct in range(CT):
                    raw = wpool.tile([P, C], F32, name=f"{nm}_raw",
                                     tag=f"{nm}_raw")
                    nc.sync.dma_start(out=raw,
                                      in_=w_d[nm][ct * P:(ct + 1) * P, :])
                    raws.append(raw)
                    wf_t = pool.tile([P, C], BF16, name=f"{nm}_f{ct}",
                                     tag=f"{nm}_f{ct}")
                    nc.scalar.mul(wf_t, raw, scales[ct])
                    wf.append(wf_t)
                keep_warm(8)
                bias_ps = pspool.tile([1, C], F32, name=f"{nm}_bps", tag="bias")
                for ct in range(CT):
                    nc.tensor.matmul(bias_ps, lhsT=b4[ct], rhs=raws[ct],
                                     start=(ct == 0), stop=(ct == CT - 1))
                bnm = "b" + nm[1:]
                braw = wpool.tile([1, C], F32, name=f"{bnm}_raw", tag="braw")
                nc.sync.dma_start(out=braw, in_=b_d[bnm][:])
                bias_sb = pmisc.tile([1, C], F32, name=f"{bnm}_sb")
                nc.vector.tensor_add(bias_sb, bias_ps, braw)
                if qscale is not None:
                    nc.vector.tensor_scalar_mul(bias_sb, bias_sb, qscale)
                keep_warm(4)
                b_pm = []
                for ct in range(CT):
                    bp_ps = pspool.tile([P, 1], F32, name=f"{bnm}_pps",
                                        tag="bias")
                    nc.tensor.matmul(bp_ps,
                                     lhsT=bias_sb[0:1, ct * P:(ct + 1) * P],
                                     rhs=one11, start=True, stop=True)
                    bp = pmisc.tile([P, 1], F32, name=f"{bnm}4_{ct}")
                    nc.vector.tensor_copy(bp, bp_ps)
                    b_pm.append(bp)
                return wf, bias_sb, b_pm

            with tc.tile_pool(name="pw", bufs=1) as pw, \
                 tc.tile_pool(name="pwraw", bufs=1) as pwraw, \
                 tc.tile_pool(name="psc", bufs=2, space="PSUM") as psc:
                wk_f, _, bk4 = fold_weight("wk", a4, None, pw, psc, pwraw)
                wq_f, _, bq4 = fold_weight(
                    "wq", aq4, 1.0 / float(np.sqrt(C)), pw, psc, pwraw)
                wv_f, bv_sb, _ = fold_weight("wv", a4, None, pw, psc, pwraw)
                bv_b = pmisc.tile([P, C], F32, name="bv_b")
                nc.gpsimd.partition_broadcast(bv_b, bv_sb)

                # ==== Phase D: projections from resident x^T ====
                # K^T[co][:, chunk] = sum_ct wk'[ct][:,co*128:] ^T @ x^T[ct]
                for ch in range(NCH):
                    for co in range(CT):
                        kps = psp.tile([P, CHUNK], F32, name="kps", tag="kps")
                        for ct in range(CT):
                            nc.tensor.matmul(
                                kps, lhsT=wk_f[ct][:, co * P:(co + 1) * P],
                                rhs=xt[ct][:, ch * CHUNK:(ch + 1) * CHUNK],
                                start=(ct == 0), stop=(ct == CT - 1))
                        nc.scalar.activation(
                            kt[co][:, ch * CHUNK:(ch + 1) * CHUNK], kps,
                            AF.Identity, bias=bk4[co], scale=1.0)
                for ch in range(QCH):
                    for co in range(CT):
                        qps = psp.tile([P, CHUNK], F32, name="qps", tag="kps")
                        for ct in range(CT):
                            nc.tensor.matmul(
                                qps, lhsT=wq_f[ct][:, co * P:(co + 1) * P],
                                rhs=xtq[ct][:, ch * CHUNK:(ch + 1) * CHUNK],
                                start=(ct == 0), stop=(ct == CT - 1))
                        nc.scalar.activation(
                            qt[co][:, ch * CHUNK:(ch + 1) * CHUNK], qps,
                            AF.Identity, bias=bq4[co], scale=1.0)
                # V natural: lhsT = x^T pixel-block, rhs = wv'
                for nt_i in range(NT):
                    vps = psp.tile([P, C], F32, name="vps", tag="kps")
                    for ct in range(CT):
                        nc.tensor.matmul(
                            vps, lhsT=xt[ct][:, nt_i * P:(nt_i + 1) * P],
                            rhs=wv_f[ct], start=(ct == 0), stop=(ct == CT - 1))
                    nc.vector.tensor_add(vv[nt_i], vps, bv_b)

        # ==== Phase E/F: attention + output projection ====
        with tc.tile_pool(name="pwo", bufs=1) as pwo, \
             tc.tile_pool(name="pres", bufs=1) as pres, \
             tc.tile_pool(name="pe", bufs=3) as pe, \
             tc.tile_pool(name="pef", bufs=2) as pef, \
             tc.tile_pool(name="pss", bufs=2, space="PSUM") as pss, \
             tc.tile_pool(name="psat", bufs=1, space="PSUM") as psat, \
             tc.tile_pool(name="psr", bufs=1, space="PSUM") as psr, \
             tc.tile_pool(name="pso", bufs=1, space="PSUM") as pso:
            wo_f = []
            for ct in range(CT):
                raw = pef.tile([P, C], F32, name="wo_raw", tag="wo_raw")
                nc.sync.dma_start(out=raw, in_=w_d["wo"][ct * P:(ct + 1) * P, :])
                wo_t = pwo.tile([P, C], BF16, name=f"wo_f{ct}", tag=f"wo_f{ct}")
                nc.scalar.copy(wo_t, raw)
                wo_f.append(wo_t)
            bo_raw = pef.tile([1, C], F32, name="bo_raw", tag="bo_raw")
            nc.sync.dma_start(out=bo_raw, in_=b_d["bo"][:])
            bo_b = pwo.tile([P, C], F32, name="bo_b", tag="bo_b")
            nc.gpsimd.partition_broadcast(bo_b, bo_raw)
            resb = []
            for i in range(QTILES):
                rraw = pef.tile([P, C], F32, name="rraw", tag="rraw")
                nc.sync.dma_start(out=rraw, in_=x_res[i * P:(i + 1) * P, :])
                rb = pres.tile([P, C], F32, name=f"resb{i}", tag=f"resb{i}")
                nc.vector.tensor_add(rb, rraw, bo_b)
                resb.append(rb)

            at_ps = [psat.tile([P, CHUNK], F32, name=f"at{i}", tag=f"at{i}")
                     for i in range(CT)]
            for qc in range(QCH):
                rows_ps = psr.tile([1, CHUNK], F32, name="rows", tag="rows")
                for kt_i in range(NT):
                    sc_ps = pss.tile([P, CHUNK], F32, name="sc", tag="sc")
                    for ct in range(CT):
                        nc.tensor.matmul(
                            sc_ps,
                            lhsT=kt[ct][:, kt_i * P:(kt_i + 1) * P],
                            rhs=qt[ct][:, qc * CHUNK:(qc + 1) * CHUNK],
                            start=(ct == 0), stop=(ct == CT - 1))
                    probs = pe.tile([P, CHUNK], BF16, name="probs", tag="probs")
                    nc.scalar.activation(probs, sc_ps, AF.Exp)
                    for co in range(CT):
                        nc.tensor.matmul(
                            at_ps[co],
                            lhsT=vv[kt_i][:, co * P:(co + 1) * P],
                            rhs=probs,
                            start=(kt_i == 0), stop=(kt_i == NT - 1))
                    nc.tensor.matmul(rows_ps, lhsT=ones_bf, rhs=probs,
                                     start=(kt_i == 0), stop=(kt_i == NT - 1))
                # softmax denominators -> per-partition reciprocals
                recip = pe.tile([1, CHUNK], F32, name="recip", tag="recip")
                nc.vector.reciprocal(recip, rows_ps)
                recip4 = []
                for qi in range(4):
                    r4_ps = psr.tile([P, 1], F32, name="r4", tag="rows")
                    nc.tensor.matmul(r4_ps,
                                     lhsT=recip[0:1, qi * P:(qi + 1) * P],
                                     rhs=one11, start=True, stop=True)
                    r4 = pe.tile([P, 1], F32, name="recip4", tag=f"recip4_{qi}")
                    nc.vector.tensor_copy(r4, r4_ps)
                    recip4.append(r4)
                # unnormalized attn^T -> SBUF (no dependency on rowsums)
                at_sb = []
                for co in range(CT):
                    a_sb = pe.tile([P, CHUNK], BF16, name="at_sb",
                                   tag=f"at_sb{co}")
                    nc.scalar.copy(a_sb, at_ps[co])
                    at_sb.append(a_sb)
                for qi in range(4):
                    ops = pso.tile([P, C], F32, name="ops", tag="ops")
                    for ct in range(CT):
                        nc.tensor.matmul(
                            ops, lhsT=at_sb[ct][:, qi * P:(qi + 1) * P],
                            rhs=wo_f[ct], start=(ct == 0), stop=(ct == CT - 1))
                    fin = pe.tile([P, C], F32, name="fin", tag="fin")
                    # normalize rows here: out_row *= 1/rowsum (per-partition)
                    nc.scalar.activation(fin, ops, AF.Copy, bias=0.0,
                                         scale=recip4[qi])
                    fin2 = pe.tile([P, C], F32, name="fin2", tag="fin2")
                    nc.vector.tensor_add(fin2, fin, resb[qc * 4 + qi])
                    r0 = (qc * 4 + qi) * P
                    nc.sync.dma_start(out=out_d[r0:r0 + P, :], in_=fin2)

    nc.compile()
    return nc


def _consts():
    ident = np.eye(P, dtype=np.float32)
    gind = np.zeros((P, 8), dtype=np.float32)
    for p in range(P):
        gind[p, p // GS] = 1.0
    gindt = np.ascontiguousarray(gind.T)
    return ident, gind, gindt


def _make_in_maps(inputs):
    import ml_dtypes
    x = np.ascontiguousarray(np.asarray(inputs["inputs"], dtype=np.float32))
    xf = x.reshape(B, N, C)
    xf_bf = xf.astype(ml_dtypes.bfloat16)
    ident, gind, gindt = _consts()
    shared = {
        "gamma": np.ascontiguousarray(np.asarray(inputs["gn_gamma"], np.float32)),
        "beta": np.ascontiguousarray(np.asarray(inputs["gn_beta"], np.float32)),
        "ident": ident, "gind": gind, "gindt": gindt,
    }
    for nm in ("wq", "wk", "wv", "wo", "bq", "bk", "bv", "bo"):
        shared[nm] = np.ascontiguousarray(np.asarray(inputs[nm], np.float32))
    in_maps = []
    for core in range(NCORES):
        b, qq = divmod(core, 4)
        xr = np.ascontiguousarray(xf[b, qq * QS:(qq + 1) * QS, :])
        m = dict(shared)
        m["x_bf"] = np.ascontiguousarray(xf_bf[b])
        m["x_resq_bf"] = np.ascontiguousarray(xf_bf[b, qq * QS:(qq + 1) * QS, :])
        m["x_res"] = xr
        in_maps.append(m)
    return in_maps


def _assemble(results):
    out = np.empty((B, N, C), dtype=np.float32)
    for core in range(NCORES):
        b, qq = divmod(core, 4)
        out[b, qq * QS:(qq + 1) * QS, :] = results[core]["out"]
    return out.reshape(B, HH, WW, C)


def kernel(**inputs):
    global _NC_CACHE
    if _NC_CACHE is None:
        _NC_CACHE = _build()
    in_maps = _make_in_maps(inputs)
    res = run_bass_kernel_spmd(_NC_CACHE, in_maps, list(range(NCORES)))
    return _assemble(res.results)


def _install_ntff_shim():
    """The agent image's antenv lacks axon_hooks; provide it so
    run_bass_kernel_spmd(trace=True) can NTFF-profile through axon."""
    import types
    import antenv
    if "antenv.axon_hooks" in sys.modules:
        return
    mod = types.ModuleType("antenv.axon_hooks")
    mod._hook = None

    def set_axon_ntff_profile_hook(h):
        mod._hook = h

    def get_axon_ntff_profile_hook():
        return mod._hook

    mod.set_axon_ntff_profile_hook = set_axon_ntff_profile_hook
    mod.get_axon_ntff_profile_hook = get_axon_ntff_profile_hook
    sys.modules["antenv.axon_hooks"] = mod
    antenv.axon_hooks = mod
    sys.path.insert(0, "/root/.axon_site")
    from trn_agent_boot.trn_boot import _ntff_profile_via_ctypes
    hook = _ntff_profile_via_ctypes("/opt/axon/libaxon_pjrt.so")
    set_axon_ntff_profile_hook(hook)


def run_traced(inputs, trace_kwargs=None):
    """Traced run for profiling: returns (BassKernelResults, tmpdir)."""
    global _NC_CACHE
    if _NC_CACHE is None:
        _NC_CACHE = _build()
    import tempfile
    _install_ntff_shim()
    in_maps = _make_in_maps(inputs)
    tmpdir = tempfile.mkdtemp(prefix="trace_")
    res = run_bass_kernel_spmd(_NC_CACHE, in_maps, list(range(NCORES)),
                               trace=True, tmpdir=tmpdir,
                               trace_kwargs=trace_kwargs or {})
    return res, tmpdir



# revision 8
# speedup vs baseline: 1.9020x; 1.9020x over previous
"""Trainium2 Bass kernel for Conv2D (1x1) multi-head attention block.

Reference computation (per batch image of [64, 64, 512] = [N=4096, C=512]):
    x  = GroupNorm(inputs, G=32, eps=1e-6) * gamma + beta
    q, k, v = x @ wq + bq, x @ wk + bk, x @ wv + bv      (1x1 convs)
    scores  = (q / sqrt(C)) @ k^T                         [N, N]
    out     = softmax(scores) @ v @ wo + bo + inputs

Sharding: 8 cores = 2 batches x 4 query-quarters. Each core holds the full
image of its batch (GroupNorm stats + full-attention K/V) and computes the
output rows of its query quarter.  No collectives.

Implementation notes:
  - x^T arrives pre-transposed from the host in fp8e4m3 "DoubleRow pair"
    layout [j, p, i, n] = x[n, 256j+128i+p]: every matmul contracts 256
    channels per instruction via MatmulPerfMode.DoubleRow (2 fp8
    weights/cell), ~1.8x the bf16 matmul rate.
  - GroupNorm is folded into the projection weights (a = gamma*rstd,
    b = beta - mean*a).  Folded weights are scaled x16 before the fp8 cast
    so their sigma~0.044 values land in e4m3's normal range (subnormal
    quantization of unscaled weights costs ~4x the end-to-end error); the
    1/16 is applied for free in the PSUM->SBUF evacuation scale.
  - Softmax bias algebra: the K projection needs NO bias at all (q.bk is
    constant along the key axis -> cancels in softmax); the V bias passes
    through the softmax average exactly -> folded into the output bias
    (bo' = bv@wo + bo), so V tiles are pure matmul outputs.  Only the Q
    bias survives (bq'.k varies per key).
  - Attention: scores^T tiles = DoubleRow matmuls of K^T pairs x Q^T;
    exp(s - 2) runs fp32 PSUM -> fp8 SBUF on the scalar engine (scores are
    O(1) by construction, max ~6, so exp stays far below e4m3's 240 max);
    probs^T pairs feed DoubleRow matmuls with V pairs accumulating
    unnormalized attn^T over all 16 key-tile pairs, plus a ones-column
    matmul for the softmax denominators.  The output projection consumes
    attn^T/256 in fp8; 16/rowsum is applied per-partition at the final
    PSUM->SBUF copy.  End-to-end rel err ~4e-3 (vs 2e-2 budget).
"""

import sys

sys.path.insert(0, "/opt/trn_rl_repo")

from contextlib import ExitStack

import numpy as np

import concourse.bacc as bacc
import concourse.tile as tile
from concourse import mybir
from concourse.bass_utils import run_bass_kernel_spmd

# Problem shape (hardcoded; kernel.py must be self-contained).
B, HH, WW, C = 2, 64, 64, 512
N = HH * WW          # 4096 pixels per batch image
G = 32               # groupnorm groups
GS = C // G          # 16 channels per group
EPS = 1e-6
P = 128              # partitions
NJ = 2               # channel pair-tiles (each pair = 256 channels)
NCORES = 8
QS = N // 4          # 1024 query rows per core
CHUNK = 512          # q-chunk width (PSUM bank limit for fp32 scores)
QCH = QS // CHUNK    # 2 query chunks per core
NPAIR = N // 256     # 16 key-tile pairs
WS = 16.0            # fp8 weight pre-scale
SHIFT = 2.0          # exp(s - SHIFT) to keep probs in e4m3 range

F32 = mybir.dt.float32
BF16 = mybir.dt.bfloat16
FP8 = mybir.dt.float8e4
AF = mybir.ActivationFunctionType
DR = mybir.MatmulPerfMode.DoubleRow

_NC_CACHE = None


def _build():
    nc = bacc.Bacc(None, target_bir_lowering=False, debug=False)

    # x^T pre-transposed on host into the DoubleRow pair layout:
    # xt8_d[j, p, i, n] = x[n, 256j + 128i + p]  (fp8e4m3)
    xt8_d = nc.dram_tensor("xt8", [NJ, P, 2, N], FP8, kind="ExternalInput")
    x_res = nc.dram_tensor("x_res", [QS, C], F32, kind="ExternalInput")
    gamma_d = nc.dram_tensor("gamma", [C], F32, kind="ExternalInput")
    beta_d = nc.dram_tensor("beta", [C], F32, kind="ExternalInput")
    w_d = {}
    for nm in ("wq", "wk", "wv", "wo"):
        w_d[nm] = nc.dram_tensor(nm, [C, C], BF16, kind="ExternalInput")
    b_d = {}
    for nm in ("bq", "bv", "bo"):
        b_d[nm] = nc.dram_tensor(nm, [C], F32, kind="ExternalInput")
    gind_d = nc.dram_tensor("gind", [P, 8], F32, kind="ExternalInput")
    gindt_d = nc.dram_tensor("gindt", [8, P], F32, kind="ExternalInput")
    out_d = nc.dram_tensor("out", [QS, C], F32, kind="ExternalOutput")

    # Every core runs the same program; the host rotates the PIXEL axis of
    # xt8 per core so this core's query quarter sits at n in [0, QS).
    # Attention sums over all keys, so key order is irrelevant.

    with tile.TileContext(nc) as tc, ExitStack() as top:
        # ---- persistent pools ----
        consts = top.enter_context(tc.tile_pool(name="consts", bufs=1))
        pxt = top.enter_context(tc.tile_pool(name="pxt", bufs=1))
        pkt = top.enter_context(tc.tile_pool(name="pkt", bufs=1))
        pqt = top.enter_context(tc.tile_pool(name="pqt", bufs=1))
        pv = top.enter_context(tc.tile_pool(name="pv", bufs=1))
        pw8 = top.enter_context(tc.tile_pool(name="pw8", bufs=1))
        pres = top.enter_context(tc.tile_pool(name="pres", bufs=1))
        pmisc = top.enter_context(tc.tile_pool(name="pmisc", bufs=1))

        # PSUM pools: 3 rotating work banks + 4 attn banks + 1 rowsum bank
        pwork = top.enter_context(tc.tile_pool(name="pwork", bufs=3,
                                               space="PSUM"))
        psat = top.enter_context(tc.tile_pool(name="psat", bufs=1,
                                              space="PSUM"))
        psr = top.enter_context(tc.tile_pool(name="psr", bufs=1, space="PSUM"))

        # ---- constants (gpsimd dispatch queue; sync is busy with x^T) ----
        gind = consts.tile([P, 8], F32, name="gind")
        nc.gpsimd.dma_start(out=gind, in_=gind_d[:])
        gindt = consts.tile([8, P], F32, name="gindt")
        nc.gpsimd.dma_start(out=gindt, in_=gindt_d[:])
        gamma4, beta4 = [], []
        for ct in range(4):
            gt_ = consts.tile([P, 1], F32, name=f"gamma4_{ct}")
            nc.gpsimd.dma_start(out=gt_, in_=gamma_d[ct * P:(ct + 1) * P])
            gamma4.append(gt_)
            bt_ = consts.tile([P, 1], F32, name=f"beta4_{ct}")
            nc.gpsimd.dma_start(out=bt_, in_=beta_d[ct * P:(ct + 1) * P])
            beta4.append(bt_)
        braw = {}
        for nm in ("bq", "bv", "bo"):
            t_ = consts.tile([1, C], F32, name=f"{nm}_raw")
            nc.gpsimd.dma_start(out=t_, in_=b_d[nm][:])
            braw[nm] = t_
        ones_f32 = consts.tile([P, 1], F32, name="ones_f32")
        nc.vector.memset(ones_f32, 1.0)
        one11 = ones_f32[0:1, 0:1]
        # 16-wide so the DoubleRow lhsT middle-dim byte step is 16 (ISA req)
        ones8_t = consts.tile([P, 2, 16], FP8, name="ones8")
        nc.vector.memset(ones8_t, 1.0)
        ones8 = ones8_t[:, :, 0:1]
        onesrow_bf = consts.tile([1, P], BF16, name="onesrow_bf")
        nc.vector.memset(onesrow_bf, 1.0)
        negshift = consts.tile([P, 1], F32, name="negshift")
        nc.vector.memset(negshift, -SHIFT)

        # ---- resident activations (all fp8 pair layout) ----
        xt8 = [pxt.tile([P, 2, N], FP8, name=f"xt8_{j}", tag=f"xt8_{j}")
               for j in range(NJ)]
        kt8 = [pkt.tile([P, 2, N], FP8, name=f"kt8_{j}", tag=f"kt8_{j}")
               for j in range(NJ)]
        qt8 = [pqt.tile([P, 2, QS], FP8, name=f"qt8_{j}", tag=f"qt8_{j}")
               for j in range(NJ)]
        vv8 = [pv.tile([P, 2, C], FP8, name=f"vv8_{i}", tag=f"vv8_{i}")
               for i in range(NPAIR)]

        # ---- raw weights (bf16, host-cast), folded weights (fp8) ----
        with ExitStack() as dphase:
            pwraw = dphase.enter_context(tc.tile_pool(name="pwraw", bufs=1))
            pb = dphase.enter_context(tc.tile_pool(name="pb", bufs=2))
            wraw = {}
            for nm in ("wk", "wq", "wv", "wo"):  # fold order
                wraw[nm] = []
                for ct in range(4):
                    t_ = pwraw.tile([P, C], BF16, name=f"{nm}_raw{ct}",
                                    tag=f"{nm}_raw{ct}")
                    nc.scalar.dma_start(out=t_,
                                        in_=w_d[nm][ct * P:(ct + 1) * P, :])
                    wraw[nm].append(t_)

            # ---- Phase A: stream x^T in, accumulate bn stats ----
            bnst = [pmisc.tile([P, 8, 6], F32, name=f"bnst{ct}")
                    for ct in range(4)]
            for j in range(NJ):
                for i in range(2):
                    for h in range(2):
                        dst = xt8[j][:, i, h * 2048:(h + 1) * 2048]
                        nc.sync.dma_start(
                            out=dst, in_=xt8_d[j, :, i, h * 2048:(h + 1) * 2048])
                        for s in range(4):
                            c0 = h * 2048 + s * 512
                            nc.vector.bn_stats(
                                out=bnst[2 * j + i][:, h * 4 + s, :],
                                in_=xt8[j][:, i, c0:c0 + 512])
                        # warm-keeper: tiny matmul reading the fresh chunk
                        jnk = pwork.tile([P, CHUNK], F32, name="jnk", tag="w")
                        nc.tensor.matmul(
                            jnk, lhsT=xt8[j][:, i, h * 2048:h * 2048 + P],
                            rhs=xt8[j][:, i, h * 2048:h * 2048 + CHUNK],
                            start=True, stop=True)

            # x_res tiles (residual; needed late -- gpsimd queue)
            xres_t = []
            for i in range(8):
                t_ = pres.tile([P, C], F32, name=f"xres{i}", tag=f"xres{i}")
                nc.gpsimd.dma_start(out=t_, in_=x_res[i * P:(i + 1) * P, :])
                xres_t.append(t_)

            # dense warm burst so the HAM clock is at 8/8 when the
            # projections start (PE is otherwise idle during stats/folds)
            for r in range(12):
                jnk = pwork.tile([P, CHUNK], F32, name="jnk", tag="w")
                nc.tensor.matmul(jnk, lhsT=xt8[1][:, 1, 0:P],
                                 rhs=xt8[1][:, 1, 0:CHUNK],
                                 start=True, stop=True)

            # ---- Phase B: group stats -> per-channel scales ----
            ak4 = []   # 16 * gamma * rstd          (K/V weight scale)
            aq4 = []   # 16 * gamma * rstd / sqrt C (Q weight scale)
            b4bf = []  # beta - mean*a  (bf16, for bias folds)
            for ct in range(4):
                mv = pb.tile([P, 2], F32, name="mv", tag="mv")
                nc.vector.bn_aggr(out=mv, in_=bnst[ct])
                me2 = pb.tile([P, 2], F32, name="me2", tag="me2")
                nc.vector.tensor_copy(me2[:, 0:1], mv[:, 0:1])
                nc.vector.tensor_mul(me2[:, 1:2], mv[:, 0:1], mv[:, 0:1])
                nc.vector.tensor_add(me2[:, 1:2], me2[:, 1:2], mv[:, 1:2])
                grp_ps = pwork.tile([8, 2], F32, name="grp_ps", tag="w")
                nc.tensor.matmul(grp_ps, lhsT=gind, rhs=me2,
                                 start=True, stop=True)
                grp = pb.tile([8, 2], F32, name="grp", tag="grp")
                nc.vector.tensor_scalar_mul(grp, grp_ps, 1.0 / GS)
                var = pb.tile([8, 1], F32, name="var", tag="var")
                nc.vector.tensor_mul(var, grp[:, 0:1], grp[:, 0:1])
                nc.vector.tensor_sub(var, grp[:, 1:2], var)
                nc.vector.tensor_scalar_add(var, var, EPS)
                rstd = pb.tile([8, 1], F32, name="rstd", tag="rstd")
                nc.vector.reciprocal(rstd, var)
                nc.scalar.sqrt(rstd, rstd)
                mr = pb.tile([8, 2], F32, name="mr", tag="mr")
                nc.vector.tensor_copy(mr[:, 0:1], grp[:, 0:1])
                nc.vector.tensor_copy(mr[:, 1:2], rstd)
                mch_ps = pwork.tile([P, 2], F32, name="mch_ps", tag="w")
                nc.tensor.matmul(mch_ps, lhsT=gindt, rhs=mr,
                                 start=True, stop=True)
                mch = pb.tile([P, 2], F32, name="mch", tag="mch")
                nc.vector.tensor_copy(mch, mch_ps)
                a_t = pmisc.tile([P, 1], F32, name=f"a4_{ct}")
                nc.vector.tensor_mul(a_t, gamma4[ct], mch[:, 1:2])
                ak_t = pmisc.tile([P, 1], F32, name=f"ak4_{ct}")
                nc.vector.tensor_scalar_mul(ak_t, a_t, WS)
                ak4.append(ak_t)
                aq_t = pmisc.tile([P, 1], F32, name=f"aq4_{ct}")
                nc.vector.tensor_scalar_mul(aq_t, a_t,
                                            WS / float(np.sqrt(C)))
                aq4.append(aq_t)
                b_t = pb.tile([P, 1], F32, name="b_t", tag="b_t")
                nc.vector.tensor_mul(b_t, mch[:, 0:1], a_t)
                nc.vector.tensor_sub(b_t, beta4[ct], b_t)
                bbf = pmisc.tile([P, 1], BF16, name=f"b4bf_{ct}")
                nc.vector.tensor_copy(bbf, b_t)
                b4bf.append(bbf)

            # ---- Phase C: fold weights to fp8 (x16), fold biases ----
            wk8, wq8, wv8, wo8 = [], [], [], []
            for j in range(NJ):
                for (lst, nm, scl) in ((wk8, "wk", ak4), (wq8, "wq", aq4),
                                       (wv8, "wv", ak4)):
                    if j == 0:
                        lst.extend(
                            pw8.tile([P, 2, C], FP8, name=f"{nm}8_{jj}",
                                     tag=f"{nm}8_{jj}") for jj in range(NJ))
                    for i in range(2):
                        nc.scalar.mul(lst[j][:, i, :], wraw[nm][2 * j + i],
                                      scl[2 * j + i])
                if j == 0:
                    wo8.extend(pw8.tile([P, 2, C], FP8, name=f"wo8_{jj}",
                                        tag=f"wo8_{jj}") for jj in range(NJ))
                for i in range(2):
                    nc.scalar.mul(wo8[j][:, i, :], wraw["wo"][2 * j + i], WS)

            # Q bias: bq' = (b @ wq_raw + bq) / sqrt(C), to per-partition
            bq_ps = pwork.tile([1, C], F32, name="bq_ps", tag="w")
            for ct in range(4):
                nc.tensor.matmul(bq_ps, lhsT=b4bf[ct], rhs=wraw["wq"][ct],
                                 start=(ct == 0), stop=(ct == 3))
            bq_sb = pmisc.tile([1, C], F32, name="bq_sb")
            nc.vector.tensor_add(bq_sb, bq_ps, braw["bq"])
            nc.vector.tensor_scalar_mul(bq_sb, bq_sb,
                                        1.0 / float(np.sqrt(C)))
            bq4 = []
            for ct in range(4):
                t_ps = pwork.tile([P, 1], F32, name="bq4_ps", tag="w")
                nc.tensor.matmul(t_ps, lhsT=bq_sb[0:1, ct * P:(ct + 1) * P],
                                 rhs=one11, start=True, stop=True)
                t_ = pmisc.tile([P, 1], F32, name=f"bq4_{ct}")
                nc.vector.tensor_copy(t_, t_ps)
                bq4.append(t_)

            # V bias folded through softmax into output bias:
            # bo' = (b @ wv_raw + bv) @ wo_raw + bo, broadcast to [P, C]
            bv_ps = pwork.tile([1, C], F32, name="bv_ps", tag="w")
            for ct in range(4):
                nc.tensor.matmul(bv_ps, lhsT=b4bf[ct], rhs=wraw["wv"][ct],
                                 start=(ct == 0), stop=(ct == 3))
            bv_sb = pmisc.tile([1, C], F32, name="bv_sb")
            nc.vector.tensor_add(bv_sb, bv_ps, braw["bv"])
            bv4bf = []
            for ct in range(4):
                t_ps = pwork.tile([P, 1], F32, name="bv4_ps", tag="w")
                nc.tensor.matmul(t_ps, lhsT=bv_sb[0:1, ct * P:(ct + 1) * P],
                                 rhs=one11, start=True, stop=True)
                t_ = pmisc.tile([P, 1], BF16, name=f"bv4bf_{ct}")
                nc.vector.tensor_copy(t_, t_ps)
                bv4bf.append(t_)
            bo2_ps = pwork.tile([1, C], F32, name="bo2_ps", tag="w")
            for ct in range(4):
                nc.tensor.matmul(bo2_ps, lhsT=bv4bf[ct], rhs=wraw["wo"][ct],
                                 start=(ct == 0), stop=(ct == 3))
            bo2_sb = pmisc.tile([1, C], BF16, name="bo2_sb")
            nc.vector.tensor_add(bo2_sb, bo2_ps, braw["bo"])
            bob_ps = pwork.tile([P, C], F32, name="bob_ps", tag="w")
            nc.tensor.matmul(bob_ps, lhsT=onesrow_bf, rhs=bo2_sb,
                             start=True, stop=True)
            bo_b = pmisc.tile([P, C], F32, name="bo_b")
            nc.vector.tensor_copy(bo_b, bob_ps)

            # ---- Phase D: projections (fp8 DoubleRow, contract 256/mm) ----
            # Q^T (this core's quarter sits at pixels [0, QS) -- host rotates)
            for ch in range(QCH):
                for co in range(4):
                    qps = pwork.tile([P, CHUNK], F32, name="qps", tag="w")
                    for j in range(NJ):
                        nc.tensor.matmul(
                            qps, lhsT=wq8[j][:, :, co * P:(co + 1) * P],
                            rhs=xt8[j][:, :, ch * CHUNK:(ch + 1) * CHUNK],
                            start=(j == 0), stop=(j == NJ - 1), perf_mode=DR)
                    nc.scalar.activation(
                        qt8[co // 2][:, co % 2, ch * CHUNK:(ch + 1) * CHUNK],
                        qps, AF.Identity, bias=bq4[co], scale=1.0 / WS)
            # K^T (no bias -- cancels in softmax)
            for ch in range(N // CHUNK):
                for co in range(4):
                    kps = pwork.tile([P, CHUNK], F32, name="kps", tag="w")
                    for j in range(NJ):
                        nc.tensor.matmul(
                            kps, lhsT=wk8[j][:, :, co * P:(co + 1) * P],
                            rhs=xt8[j][:, :, ch * CHUNK:(ch + 1) * CHUNK],
                            start=(j == 0), stop=(j == NJ - 1), perf_mode=DR)
                    nc.scalar.mul(
                        kt8[co // 2][:, co % 2, ch * CHUNK:(ch + 1) * CHUNK],
                        kps, 1.0 / WS)
            # V natural (no bias -- folded into bo'); pixel-tile lhsT
            for nt in range(N // P):
                vps = pwork.tile([P, C], F32, name="vps", tag="w")
                for j in range(NJ):
                    nc.tensor.matmul(
                        vps, lhsT=xt8[j][:, :, nt * P:(nt + 1) * P],
                        rhs=wv8[j], start=(j == 0), stop=(j == NJ - 1),
                        perf_mode=DR)
                nc.vector.tensor_scalar_mul(vv8[nt // 2][:, nt % 2, :], vps,
                                            1.0 / WS)

            # residual + output bias tiles (DVE, off critical path)
            resb = []
            for i in range(8):
                t_ = pres.tile([P, C], F32, name=f"resb{i}", tag=f"resb{i}")
                nc.vector.tensor_add(t_, xres_t[i], bo_b)
                resb.append(t_)

        # ---- Phase E/F: attention + output projection ----
        with tc.tile_pool(name="pe", bufs=3) as pe, \
             tc.tile_pool(name="pf", bufs=2) as pf:
            at_ps = [psat.tile([P, CHUNK], F32, name=f"at{i}", tag=f"at{i}")
                     for i in range(4)]

            def emit_sc(qc, pair):
                """scores + exp for one key-tile pair -> probs8 tile"""
                probs = pe.tile([P, 2, CHUNK], FP8, name="probs", tag="probs")
                for i in range(2):
                    kt_i = 2 * pair + i
                    sc_ps = pwork.tile([P, CHUNK], F32, name="sc", tag="w")
                    for j in range(NJ):
                        nc.tensor.matmul(
                            sc_ps,
                            lhsT=kt8[j][:, :, kt_i * P:(kt_i + 1) * P],
                            rhs=qt8[j][:, :, qc * CHUNK:(qc + 1) * CHUNK],
                            start=(j == 0), stop=(j == NJ - 1), perf_mode=DR)
                    nc.scalar.activation(probs[:, i, :], sc_ps, AF.Exp,
                                         bias=negshift)
                return probs

            def emit_at(pair, probs, rows_ps):
                for co in range(4):
                    nc.tensor.matmul(
                        at_ps[co], lhsT=vv8[pair][:, :, co * P:(co + 1) * P],
                        rhs=probs, start=(pair == 0), stop=(pair == NPAIR - 1),
                        perf_mode=DR)
                nc.tensor.matmul(rows_ps, lhsT=ones8, rhs=probs,
                                 start=(pair == 0), stop=(pair == NPAIR - 1),
                                 perf_mode=DR)

            for qc in range(QCH):
                rows_ps = psr.tile([1, CHUNK], F32, name="rows", tag="rows")
                probs_prev = None
                for pair in range(NPAIR):
                    probs = emit_sc(qc, pair)
                    if probs_prev is not None:
                        emit_at(pair - 1, probs_prev, rows_ps)
                    probs_prev = probs
                emit_at(NPAIR - 1, probs_prev, rows_ps)

                # softmax denominators -> per-partition 16/rowsum
                rows_sb = pe.tile([1, CHUNK], F32, name="rows_sb",
                                  tag="rows_sb")
                nc.vector.tensor_copy(rows_sb, rows_ps)
                recip4 = []
                for qi in range(4):
                    r_ps = psr.tile([P, 1], F32, name="r4", tag="rows")
                    nc.tensor.matmul(r_ps,
                                     lhsT=rows_sb[0:1, qi * P:(qi + 1) * P],
                                     rhs=one11, start=True, stop=True)
                    r_ = pe.tile([P, 1], F32, name="recip4", tag=f"recip{qi}")
                    nc.vector.tensor_scalar_mul(r_, r_ps, 1.0 / WS)
                    nc.vector.reciprocal(r_, r_)
                    recip4.append(r_)
                # unnormalized attn^T -> fp8 (/256), on DVE (scalar is
                # saturated with exp; DVE evac also avoids a FIFO cycle
                # with the next qc's attention)
                at8 = [pe.tile([P, 2, CHUNK], FP8, name=f"at8_{j}",
                               tag=f"at8_{j}") for j in range(NJ)]
                for co in range(4):
                    nc.vector.tensor_scalar_mul(at8[co // 2][:, co % 2, :],
                                                at_ps[co], 1.0 / 256.0)
                for qi in range(4):
                    ops = pwork.tile([P, C], F32, name="ops", tag="w")
                    for j in range(NJ):
                        nc.tensor.matmul(
                            ops, lhsT=at8[j][:, :, qi * P:(qi + 1) * P],
                            rhs=wo8[j], start=(j == 0), stop=(j == NJ - 1),
                            perf_mode=DR)
                    # normalize rows here: out_row *= 16/rowsum; +resb after.
                    # (at8 = attn_unnorm/256, wo8 = 16*wo ->
                    #  ops = attn_unnorm @ wo / 16; want /rowsum * 16)
                    fin = pf.tile([P, C], F32, name="fin", tag="fin")
                    nc.scalar.activation(fin, ops, AF.Copy, bias=0.0,
                                         scale=recip4[qi])
                    fin2 = pf.tile([P, C], F32, name="fin2", tag="fin2")
                    nc.vector.tensor_add(fin2, fin, resb[qc * 4 + qi])
                    r0 = (qc * 4 + qi) * P
                    nc.sync.dma_start(out=out_d[r0:r0 + P, :], in_=fin2)

    nc.compile()
    return nc


def _consts():
    gind = np.zeros((P, 8), dtype=np.float32)
    for p in range(P):
        gind[p, p // GS] = 1.0
    gindt = np.ascontiguousarray(gind.T)
    return gind, gindt


def _make_in_maps(inputs):
    import ml_dtypes
    x = np.ascontiguousarray(np.asarray(inputs["inputs"], dtype=np.float32))
    xf = x.reshape(B, N, C)
    gind, gindt = _consts()
    shared = {
        "gamma": np.ascontiguousarray(np.asarray(inputs["gn_gamma"], np.float32)),
        "beta": np.ascontiguousarray(np.asarray(inputs["gn_beta"], np.float32)),
        "gind": gind, "gindt": gindt,
    }
    for nm in ("wq", "wk", "wv", "wo"):
        shared[nm] = np.ascontiguousarray(
            np.asarray(inputs[nm], np.float32).astype(ml_dtypes.bfloat16))
    for nm in ("bq", "bv", "bo"):
        shared[nm] = np.ascontiguousarray(np.asarray(inputs[nm], np.float32))
    # x^T in fp8 pair layout per batch: [j, p, i, n] = x[n, 256j+128i+p]
    xt8 = {}
    for b in range(B):
        t = xf[b].T.astype(ml_dtypes.float8_e4m3)           # [C, N]
        xt8[b] = np.ascontiguousarray(t.reshape(NJ, 2, P, N).transpose(0, 2, 1, 3))
    in_maps = []
    for core in range(NCORES):
        b, qq = divmod(core, 4)
        m = dict(shared)
        # rotate pixels so this core's query quarter sits at n in [0, QS)
        m["xt8"] = np.ascontiguousarray(
            np.roll(xt8[b], -qq * QS, axis=3))
        m["x_res"] = np.ascontiguousarray(xf[b, qq * QS:(qq + 1) * QS, :])
        in_maps.append(m)
    return in_maps


def _assemble(results):
    out = np.empty((B, N, C), dtype=np.float32)
    for core in range(NCORES):
        b, qq = divmod(core, 4)
        out[b, qq * QS:(qq + 1) * QS, :] = results[core]["out"]
    return out.reshape(B, HH, WW, C)


def kernel(**inputs):
    global _NC_CACHE
    if _NC_CACHE is None:
        _NC_CACHE = _build()
    in_maps = _make_in_maps(inputs)
    res = run_bass_kernel_spmd(_NC_CACHE, in_maps, list(range(NCORES)))
    return _assemble(res.results)


def _install_ntff_shim():
    """The agent image's antenv lacks axon_hooks; provide it so
    run_bass_kernel_spmd(trace=True) can NTFF-profile through axon."""
    import types
    import antenv
    if "antenv.axon_hooks" in sys.modules:
        return
    mod = types.ModuleType("antenv.axon_hooks")
    mod._hook = None

    def set_axon_ntff_profile_hook(h):
        mod._hook = h

    def get_axon_ntff_profile_hook():
        return mod._hook

    mod.set_axon_ntff_profile_hook = set_axon_ntff_profile_hook
    mod.get_axon_ntff_profile_hook = get_axon_ntff_profile_hook
    sys.modules["antenv.axon_hooks"] = mod
    antenv.axon_hooks = mod
    sys.path.insert(0, "/root/.axon_site")
    from trn_agent_boot.trn_boot import _ntff_profile_via_ctypes
    hook = _ntff_profile_via_ctypes("/opt/axon/libaxon_pjrt.so")
    set_axon_ntff_profile_hook(hook)


def run_traced(inputs, trace_kwargs=None):
    """Traced run for profiling: returns (BassKernelResults, tmpdir)."""
    global _NC_CACHE
    if _NC_CACHE is None:
        _NC_CACHE = _build()
    import tempfile
    _install_ntff_shim()
    in_maps = _make_in_maps(inputs)
    tmpdir = tempfile.mkdtemp(prefix="trace_")
    res = run_bass_kernel_spmd(_NC_CACHE, in_maps, list(range(NCORES)),
                               trace=True, tmpdir=tmpdir,
                               trace_kwargs=trace_kwargs or {})
    return res, tmpdir
